# revision 7
# baseline (speedup 1.0000x reference)
# Trainium2 Bass kernel for nn_CNN3_F_P (pairwise conv + 3x conv1d + 2 FC).
# Data parallel over 8 NeuronCores: batch 2048 -> 256 samples/core.
# Self-contained: hardcodes all shapes; host preps DMA-friendly weight layouts.
import sys

import numpy as np

try:
    import concourse.bass as bass  # noqa: F401
except ImportError:
    sys.path.insert(0, "/opt/trn_rl_repo")

import ml_dtypes

import concourse.bass as bass
import concourse.mybir as mybir
import concourse.tile as tile
from concourse import bacc
from concourse.bass_utils import run_bass_kernel_spmd

# Problem shapes
INST, CTX = 64, 128
PC = 256          # pairwise out channels; CH1=CH2=CH3=256
L = CTX - 1       # 127
F1, OUT = 400, 2
B = 2048
N_CORES = 8
BPC = B // N_CORES  # 256 samples per core
GT = 4              # samples per matmul group (free dim GT*L = 508 <= 512)

FP32 = mybir.dt.float32
BF16 = mybir.dt.bfloat16
BF16_NP = ml_dtypes.bfloat16
RELU = mybir.ActivationFunctionType.Relu
IDENT = mybir.ActivationFunctionType.Identity

# f1 partition blocks (400 = 128+128+128+16)
F1_BLKS = [128, 128, 128, 16]


def build_nc(n_samples: int, debug: bool = False) -> bass.Bass:
    """Emit the per-core Tile program. Every core runs this same program on
    its own 'n_samples'-sample shard."""
    assert n_samples % GT == 0
    n_groups = n_samples // GT

    nc = bacc.Bacc()

    # DRAM parameters (per-core shard + replicated weights)
    xt_d = nc.declare_dram_parameter("xt", [INST, n_samples, CTX], BF16, isOutput=False)
    wp_d = nc.declare_dram_parameter("wp", [INST, 2, PC], BF16, isOutput=False)
    wcv_d = nc.declare_dram_parameter("wcv", [128, 3, 2, 3, 2, 128], BF16, isOutput=False)
    wf1_d = nc.declare_dram_parameter("wf1", [L, 128, 2, F1], BF16, isOutput=False)
    wf2_d = nc.declare_dram_parameter("wf2", [128, 4, OUT], BF16, isOutput=False)
    bcv_d = nc.declare_dram_parameter("bcv", [128, 8], FP32, isOutput=False)
    bf1_d = nc.declare_dram_parameter("bf1", [128, 4], FP32, isOutput=False)
    bf2_d = nc.declare_dram_parameter("bf2", [OUT, 1], FP32, isOutput=False)
    out_d = nc.declare_dram_parameter("out", [OUT, n_samples], FP32, isOutput=True)
    if debug:
        dbg_h = [
            nc.declare_dram_parameter(f"dbg_h{i}", [2, 128, GT, 129], FP32, isOutput=True)
            for i in range(3)
        ]
        dbg_h3 = nc.declare_dram_parameter("dbg_h3", [2, 128, n_samples, L], FP32, isOutput=True)
        dbg_f1 = nc.declare_dram_parameter("dbg_f1", [4, 128, n_samples], FP32, isOutput=True)
        dbg_x0b = nc.declare_dram_parameter("dbg_x0b", [INST, GT, L], FP32, isOutput=True)

    with tile.TileContext(nc) as tc:
        with (
            tc.tile_pool(name="consts", bufs=1) as consts,
            tc.tile_pool(name="hbuf", bufs=1) as hbuf,
            tc.tile_pool(name="xin", bufs=3) as xin,
            tc.tile_pool(name="x0b", bufs=2) as x0bp,
        ):
            # ---- resident weights/biases ----
            wp_t = consts.tile([INST, 2, PC], BF16, tag="wp", name="wp")
            nc.sync.dma_start(wp_t[:], wp_d[:])
            wcv_t = consts.tile([128, 3, 2, 3, 2, 128], BF16, tag="wcv", name="wcv")
            nc.sync.dma_start(wcv_t[:], wcv_d[:])
            wf2_t = consts.tile([128, 4, OUT], BF16, tag="wf2", name="wf2")
            nc.sync.dma_start(wf2_t[:], wf2_d[:])
            bcv_t = consts.tile([128, 8], FP32, tag="bcv", name="bcv")
            nc.sync.dma_start(bcv_t[:], bcv_d[:])
            bf1_t = consts.tile([128, 4], FP32, tag="bf1", name="bf1")
            nc.sync.dma_start(bf1_t[:], bf1_d[:])
            bf2_t = consts.tile([OUT, 1], FP32, tag="bf2", name="bf2")
            nc.sync.dma_start(bf2_t[:], bf2_d[:])

            # ---- persistent activation buffers ----
            # h0..h2: ping-pong per group parity; stripes of 129 cols/sample
            # (col 0 and col 128 are zero pads for the k=3 conv taps).
            hconv = []  # hconv[layer][parity][blk]
            for layer in range(3):
                byp = []
                for par in range(2):
                    blks = []
                    for o in range(2):
                        t = hbuf.tile([128, GT, 129], BF16, tag=f"h{layer}_{par}_{o}", name=f"h{layer}_{par}_{o}")
                        nc.vector.memset(t[:, :, 0:1], 0.0)
                        nc.vector.memset(t[:, :, 128:129], 0.0)
                        blks.append(t)
                    byp.append(blks)
                hconv.append(byp)
            # h3: conv3 output for the whole shard, dense (no pads), bf16
            h3 = [
                hbuf.tile([128, n_samples, L], BF16, tag=f"h3_{o}", name=f"h3_{o}") for o in range(2)
            ]

            # ---- phase A: pairwise + conv1..conv3 ----
            with tc.tile_pool(name="cpsum", bufs=8, space=bass.MemorySpace.PSUM) as cp:
                for g in range(n_groups):
                    par = g % 2
                    s0 = g * GT
                    xt_t = xin.tile([INST, GT, CTX], BF16, tag="xt", name="xt_t")
                    nc.sync.dma_start(xt_t[:], xt_d[:, s0 : s0 + GT, :])
                    # broadcast x(:, :, 0) across the 127 output positions
                    x0b = x0bp.tile([INST, GT, L], BF16, tag="x0b", name="x0b")
                    nc.vector.tensor_copy(
                        x0b[:], xt_t[:, :, 0:1].broadcast_to([INST, GT, L])
                    )
                    # pairwise layer -> h0
                    for o in range(2):
                        ps = cp.tile([128, GT, L], FP32, tag="cp", name="cp")
                        nc.tensor.matmul(
                            ps[:],
                            wp_t[:, 1, o * 128 : (o + 1) * 128],
                            xt_t[:, :, 1:CTX],
                            start=True,
                            stop=False,
                        )
                        nc.tensor.matmul(
                            ps[:],
                            wp_t[:, 0, o * 128 : (o + 1) * 128],
                            x0b[:],
                            start=False,
                            stop=True,
                        )
                        nc.scalar.activation(
                            hconv[0][par][o][:, :, 1:128],
                            ps[:],
                            RELU,
                            bias=bcv_t[:, o : o + 1],
                        )
                    # conv1, conv2 -> h1, h2
                    for li in range(2):
                        for o in range(2):
                            ps = cp.tile([128, GT, L], FP32, tag="cp", name="cp")
                            n_mm = 0
                            for i in range(2):
                                for k in range(3):
                                    nc.tensor.matmul(
                                        ps[:],
                                        wcv_t[:, li, i, k, o, :],
                                        hconv[li][par][i][:, :, k : k + L],
                                        start=(n_mm == 0),
                                        stop=(n_mm == 5),
                                    )
                                    n_mm += 1
                            nc.scalar.activation(
                                hconv[li + 1][par][o][:, :, 1:128],
                                ps[:],
                                RELU,
                                bias=bcv_t[:, 2 * (li + 1) + o : 2 * (li + 1) + o + 1],
                            )
                    # conv3 -> h3 (dense, full-shard buffer)
                    for o in range(2):
                        ps = cp.tile([128, GT, L], FP32, tag="cp", name="cp")
                        n_mm = 0
                        for i in range(2):
                            for k in range(3):
                                nc.tensor.matmul(
                                    ps[:],
                                    wcv_t[:, 2, i, k, o, :],
                                    hconv[2][par][i][:, :, k : k + L],
                                    start=(n_mm == 0),
                                    stop=(n_mm == 5),
                                )
                                n_mm += 1
                        nc.scalar.activation(
                            h3[o][:, s0 : s0 + GT, :],
                            ps[:],
                            RELU,
                            bias=bcv_t[:, 6 + o : 7 + o],
                        )

            # ---- phase B: fc1 (+relu) and fc2 ----
            with (
                tc.tile_pool(name="fpsum", bufs=1, space=bass.MemorySpace.PSUM) as fp,
                tc.tile_pool(name="wstream", bufs=4) as ws,
                tc.tile_pool(name="fout", bufs=1) as fo,
            ):
                f1ps = [
                    fp.tile([F1_BLKS[j], n_samples], FP32, tag=f"f1p{j}", name=f"f1p{j}")
                    for j in range(4)
                ]
                for l in range(L):
                    wt = ws.tile([128, 2, F1], BF16, tag="wf1", name="wf1_t")
                    nc.sync.dma_start(wt[:], wf1_d[l])
                    for i in range(2):
                        for j in range(4):
                            nc.tensor.matmul(
                                f1ps[j][:],
                                wt[:, i, j * 128 : j * 128 + F1_BLKS[j]],
                                h3[i][:, :, l : l + 1],
                                start=(l == 0 and i == 0),
                                stop=(l == L - 1 and i == 1),
                            )
                f1out = [
                    fo.tile([F1_BLKS[j], n_samples], BF16, tag=f"f1o{j}", name=f"f1o{j}")
                    for j in range(4)
                ]
                for j in range(4):
                    nc.scalar.activation(
                        f1out[j][:], f1ps[j][:], RELU, bias=bf1_t[: F1_BLKS[j], j : j + 1]
                    )
                f2ps = fp.tile([OUT, n_samples], FP32, tag="f2p", name="f2p")
                for j in range(4):
                    nc.tensor.matmul(
                        f2ps[:],
                        wf2_t[: F1_BLKS[j], j, :],
                        f1out[j][:],
                        start=(j == 0),
                        stop=(j == 3),
                    )
                out_t = fo.tile([OUT, n_samples], FP32, tag="out", name="out_t")
                nc.scalar.activation(out_t[:], f2ps[:], IDENT, bias=bf2_t[:])
                nc.sync.dma_start(out_d[:], out_t[:])
                if debug:
                    for li in range(3):
                        for o in range(2):
                            dh = fo.tile([128, GT, 129], FP32, tag=f"dbg{li}{o}", name=f"dbg{li}{o}")
                            nc.vector.tensor_copy(dh[:], hconv[li][0][o][:])
                            nc.sync.dma_start(dbg_h[li][o], dh[:])
                    for o in range(2):
                        dh3 = fo.tile([128, n_samples, L], FP32, tag=f"dbgh3{o}", name=f"dbgh3{o}")
                        nc.vector.tensor_copy(dh3[:], h3[o][:])
                        nc.sync.dma_start(dbg_h3[o], dh3[:])
                    for j in range(4):
                        df = fo.tile([F1_BLKS[j], n_samples], FP32, tag=f"dbgf{j}", name=f"dbgf{j}")
                        nc.vector.tensor_copy(df[:], f1out[j][:])
                        nc.sync.dma_start(dbg_f1[j, : F1_BLKS[j], :], df[:])
                    dx = fo.tile([INST, GT, L], FP32, tag="dbgx", name="dbgx")
                    nc.vector.tensor_copy(dx[:], x0b[:])
                    nc.sync.dma_start(dbg_x0b[:], dx[:])

    nc.compile()
    return nc


def prep_inputs(x, Wp, bp, W1, b1, W2, b2, W3, b3, Wfc1, bfc1, Wfc2, bfc2):
    """Host-side layout prep (numpy). Returns dict of full-size arrays keyed
    by the kernel's DRAM parameter names; 'xt' still has the full batch."""
    f32 = np.float32
    # x: (B, CTX*INST) -> (INST, B, CTX), contiguous in CTX per (c, b) row
    xt = np.ascontiguousarray(
        x.reshape(B, CTX, INST).transpose(2, 0, 1)
    ).astype(BF16_NP)
    # Wp: (PC, INST, 2) -> (INST, 2, PC)
    wp = np.ascontiguousarray(Wp.transpose(1, 2, 0)).astype(BF16_NP)
    # conv weights: (Cout, Cin, K) -> [cin_in, layer, cin_blk, k, cout_blk, cout_in]
    def conv_t(W):
        A = W.reshape(2, 128, 2, 128, 3)  # [ob, oi, ib, ii, k]
        return A.transpose(3, 2, 4, 0, 1)  # (128, 2, 3, 2, 128)

    wcv = np.ascontiguousarray(
        np.stack([conv_t(W1), conv_t(W2), conv_t(W3)], axis=1)
    ).astype(BF16_NP)
    # Wfc1: (400, 32512) with col = c3*L + l -> (L, cin_in, cin_blk, 400)
    wf1 = np.ascontiguousarray(
        Wfc1.reshape(F1, 2, 128, L).transpose(3, 2, 1, 0)
    ).astype(BF16_NP)
    # Wfc2: (2, 400) -> zero-pad to 512 -> (128, 4, 2)
    wf2p = np.zeros((OUT, 512), dtype=f32)
    wf2p[:, :F1] = Wfc2
    wf2 = np.ascontiguousarray(wf2p.reshape(OUT, 4, 128).transpose(2, 1, 0)).astype(
        BF16_NP
    )
    # conv biases: (128, 8) fp32, col = layer*2 + blk
    bcv = np.ascontiguousarray(
        np.stack([bp, b1, b2, b3]).reshape(4, 2, 128).transpose(2, 0, 1).reshape(128, 8)
    ).astype(f32)
    bf1p = np.zeros((512,), dtype=f32)
    bf1p[:F1] = bfc1
    bf1 = np.ascontiguousarray(bf1p.reshape(4, 128).T).astype(f32)
    bf2 = np.ascontiguousarray(bfc2.reshape(OUT, 1)).astype(f32)
    return {
        "xt": xt,
        "wp": wp,
        "wcv": wcv,
        "wf1": wf1,
        "wf2": wf2,
        "bcv": bcv,
        "bf1": bf1,
        "bf2": bf2,
    }


_NC_CACHE = {}


def _get_nc(n_samples):
    if n_samples not in _NC_CACHE:
        _NC_CACHE[n_samples] = build_nc(n_samples)
    return _NC_CACHE[n_samples]


def run(inputs: dict, trace: bool = False, tmpdir: str | None = None):
    """Run on the 8 NeuronCores. Returns (output (B,2) fp32, exec_time_ns|None)."""
    full = prep_inputs(**inputs)
    xt = full.pop("xt")
    in_maps = []
    for c in range(N_CORES):
        m = dict(full)
        m["xt"] = np.ascontiguousarray(xt[:, c * BPC : (c + 1) * BPC, :])
        in_maps.append(m)
    nc = _get_nc(BPC)
    res = run_bass_kernel_spmd(
        nc,
        in_maps,
        list(range(N_CORES)),
        trace=trace,
        trace_cores=[0] if trace else None,
        tmpdir=tmpdir,
    )
    out = np.concatenate([np.asarray(r["out"]).T for r in res.results], axis=0)
    return out.astype(np.float32), res.exec_time_ns


def kernel(**inputs) -> np.ndarray:
    return run(inputs, trace=False)[0]


# revision 10
# speedup vs baseline: 1.5535x; 1.5535x over previous
# Trainium2 Bass kernel for nn_CNN3_F_P (pairwise conv + 3x conv1d + 2 FC).
# Data parallel over 8 NeuronCores: batch 2048 -> 256 samples/core.
# Self-contained: hardcodes all shapes; host preps DMA-friendly weight layouts.
import sys

import numpy as np

try:
    import concourse.bass as bass  # noqa: F401
except ImportError:
    sys.path.insert(0, "/opt/trn_rl_repo")

import ml_dtypes

import concourse.bass as bass
import concourse.mybir as mybir
import concourse.tile as tile
from concourse import bacc
from concourse.bass_utils import run_bass_kernel_spmd

# Problem shapes
INST, CTX = 64, 128
PC = 256          # pairwise out channels; CH1=CH2=CH3=256
L = CTX - 1       # 127
F1, OUT = 400, 2
B = 2048
N_CORES = 8
BPC = B // N_CORES  # 256 samples per core
GT = 4              # samples per matmul group (free dim GT*L = 508 <= 512)

FP32 = mybir.dt.float32
BF16 = mybir.dt.bfloat16
BF16_NP = ml_dtypes.bfloat16
RELU = mybir.ActivationFunctionType.Relu
ADD = mybir.AluOpType.add
MULT = mybir.AluOpType.mult


def build_nc(n_samples: int, debug: bool = False) -> bass.Bass:
    """Emit the per-core Tile program. Every core runs this same program on
    its own 'n_samples'-sample shard."""
    assert n_samples % (2 * GT) == 0
    n_groups = n_samples // GT
    sb_n = n_samples // 128 if n_samples >= 128 else 1
    sb_sz = min(n_samples, 128)

    nc = bacc.Bacc()

    # DRAM parameters (per-core shard + replicated weights).
    # xt rows 0..63 = x transposed to (inst, sample, pos); rows 64..127 = the
    # pos-0 column broadcast along pos (so the pairwise layer is one matmul).
    xt_d = nc.declare_dram_parameter("xt", [128, n_samples, CTX], BF16, isOutput=False)
    wp_d = nc.declare_dram_parameter("wp", [128, PC], BF16, isOutput=False)
    wcv_d = nc.declare_dram_parameter("wcv", [128, 3, 2, 3, 2, 128], BF16, isOutput=False)
    wf1_d = nc.declare_dram_parameter("wf1", [L, 128, 2, F1], BF16, isOutput=False)
    bf1_d = nc.declare_dram_parameter("bf1", [1, F1], BF16, isOutput=False)
    wf2_d = nc.declare_dram_parameter("wf2", [128, OUT, F1], BF16, isOutput=False)
    bcv_d = nc.declare_dram_parameter("bcv", [128, 8], FP32, isOutput=False)
    bf2_d = nc.declare_dram_parameter("bf2", [128, OUT], FP32, isOutput=False)
    out_d = nc.declare_dram_parameter("out", [n_samples, OUT], FP32, isOutput=True)
    if debug:
        dbg_h = [
            nc.declare_dram_parameter(f"dbg_h{i}", [2, 128, GT, 129], FP32, isOutput=True)
            for i in range(3)
        ]
        dbg_h3 = nc.declare_dram_parameter("dbg_h3", [2, 128, n_samples, L], FP32, isOutput=True)
        dbg_f1 = nc.declare_dram_parameter("dbg_f1", [sb_n, 128, F1], FP32, isOutput=True)

    with tile.TileContext(nc) as tc:
        with (
            tc.tile_pool(name="consts", bufs=1) as consts,
            tc.tile_pool(name="hbuf", bufs=1) as hbuf,
            tc.tile_pool(name="xin", bufs=3) as xin,
        ):
            # ---- resident weights/biases ----
            wp_t = consts.tile([128, PC], BF16, tag="wp", name="wp")
            nc.sync.dma_start(wp_t[:], wp_d[:])
            wcv_t = consts.tile([128, 3, 2, 3, 2, 128], BF16, tag="wcv", name="wcv")
            nc.sync.dma_start(wcv_t[:], wcv_d[:])
            wf2_t = consts.tile([128, OUT, F1], BF16, tag="wf2", name="wf2")
            nc.sync.dma_start(wf2_t[:], wf2_d[:])
            bcv_t = consts.tile([128, 8], FP32, tag="bcv", name="bcv")
            nc.sync.dma_start(bcv_t[:], bcv_d[:])
            bf1_t = consts.tile([1, F1], BF16, tag="bf1", name="bf1")
            nc.sync.dma_start(bf1_t[:], bf1_d[:])
            bf2_t = consts.tile([128, OUT], FP32, tag="bf2", name="bf2")
            nc.sync.dma_start(bf2_t[:], bf2_d[:])
            ones_t = consts.tile([1, 128], BF16, tag="ones", name="ones")
            nc.vector.memset(ones_t[:], 1.0)

            # ---- persistent activation buffers ----
            # h0..h2: ping-pong per group parity; stripes of 129 cols/sample
            # (col 0 and col 128 are zero pads for the k=3 conv taps).
            hconv = []  # hconv[layer][parity][blk]
            for layer in range(3):
                byp = []
                for par in range(2):
                    blks = []
                    for o in range(2):
                        t = hbuf.tile(
                            [128, GT, 129], BF16,
                            tag=f"h{layer}_{par}_{o}", name=f"h{layer}_{par}_{o}",
                        )
                        nc.vector.memset(t[:, :, 0:1], 0.0)
                        nc.vector.memset(t[:, :, 128:129], 0.0)
                        blks.append(t)
                    byp.append(blks)
                hconv.append(byp)
            # h3: conv3 output for the whole shard, dense (no pads), bf16
            h3 = [
                hbuf.tile([128, n_samples, L], BF16, tag=f"h3_{o}", name=f"h3_{o}")
                for o in range(2)
            ]

            # ---- phase A: pairwise + conv1..conv3 ----
            with tc.tile_pool(name="cpsum", bufs=8, space=bass.MemorySpace.PSUM) as cp:
                for g in range(n_groups):
                    par = g % 2
                    s0 = g * GT
                    px = xin.tile([128, GT, CTX], BF16, tag="px", name="px")
                    nc.sync.dma_start(px[:], xt_d[:, s0 : s0 + GT, :])
                    # pairwise layer -> h0 (single matmul per out-block:
                    # rows 0..63 slide over positions, rows 64..127 see x0)
                    for o in range(2):
                        ps = cp.tile([128, GT, L], FP32, tag="cp", name="cp")
                        nc.tensor.matmul(
                            ps[:],
                            wp_t[:, o * 128 : (o + 1) * 128],
                            px[:, :, 1:CTX],
                            start=True,
                            stop=True,
                        )
                        nc.scalar.activation(
                            hconv[0][par][o][:, :, 1:128],
                            ps[:],
                            RELU,
                            bias=bcv_t[:, o : o + 1],
                        )
                    # conv1..conv3
                    for li in range(3):
                        for o in range(2):
                            ps = cp.tile([128, GT, L], FP32, tag="cp", name="cp")
                            n_mm = 0
                            for i in range(2):
                                for k in range(3):
                                    nc.tensor.matmul(
                                        ps[:],
                                        wcv_t[:, li, i, k, o, :],
                                        hconv[li][par][i][:, :, k : k + L],
                                        start=(n_mm == 0),
                                        stop=(n_mm == 5),
                                    )
                                    n_mm += 1
                            if li < 2:
                                dst = hconv[li + 1][par][o][:, :, 1:128]
                            else:
                                dst = h3[o][:, s0 : s0 + GT, :]
                            nc.scalar.activation(
                                dst,
                                ps[:],
                                RELU,
                                bias=bcv_t[:, 2 * (li + 1) + o : 2 * (li + 1) + o + 1],
                            )

            # ---- phase B: fc1 (+relu) and fc2 ----
            # fc1 runs "flipped": stationary = h3 sample-block columns,
            # moving = streamed Wfc1 rows -> psum[sample, f1].
            with (
                tc.tile_pool(name="fpsum", bufs=1, space=bass.MemorySpace.PSUM) as fp,
                tc.tile_pool(name="wstream", bufs=12) as ws,
                tc.tile_pool(name="fout", bufs=1) as fo,
            ):
                f1ps = [
                    fp.tile([sb_sz, F1], FP32, tag=f"f1p{sb}", name=f"f1p{sb}")
                    for sb in range(sb_n)
                ]
                # bias row via a K=1 matmul of ones^T x bfc1
                for sb in range(sb_n):
                    nc.tensor.matmul(
                        f1ps[sb][:],
                        ones_t[:, :sb_sz],
                        bf1_t[:],
                        start=True,
                        stop=False,
                    )
                for l in range(L):
                    wt = ws.tile([128, 2, F1], BF16, tag="wf1", name="wf1_t")
                    nc.sync.dma_start(wt[:], wf1_d[l])
                    for i in range(2):
                        for sb in range(sb_n):
                            nc.tensor.matmul(
                                f1ps[sb][:],
                                h3[i][:, sb * 128 : sb * 128 + sb_sz, l],
                                wt[:, i, :],
                                start=False,
                                stop=(l == L - 1 and i == 1),
                            )
                for sb in range(sb_n):
                    f1o = fo.tile([sb_sz, F1], BF16, tag=f"f1o{sb}", name=f"f1o{sb}")
                    nc.scalar.activation(f1o[:], f1ps[sb][:], RELU)
                    out_t = fo.tile([sb_sz, OUT], FP32, tag=f"out{sb}", name=f"out{sb}")
                    for o in range(OUT):
                        tmp = fo.tile([sb_sz, F1], FP32, tag="tmp", name="tmp", bufs=2)
                        nc.vector.tensor_tensor(tmp[:], f1o[:], wf2_t[:sb_sz, o, :], MULT)
                        nc.vector.tensor_reduce(
                            out_t[:, o : o + 1], tmp[:], mybir.AxisListType.X, ADD
                        )
                    nc.vector.tensor_tensor(out_t[:], out_t[:], bf2_t[:sb_sz, :], ADD)
                    nc.sync.dma_start(out_d[sb * 128 : sb * 128 + sb_sz, :], out_t[:])
                    if debug:
                        df = fo.tile([sb_sz, F1], FP32, tag=f"dbgf{sb}", name=f"dbgf{sb}")
                        nc.vector.tensor_copy(df[:], f1o[:])
                        nc.sync.dma_start(dbg_f1[sb, :sb_sz, :], df[:])
                if debug:
                    for li in range(3):
                        for o in range(2):
                            dh = fo.tile([128, GT, 129], FP32, tag=f"dbg{li}{o}", name=f"dbg{li}{o}")
                            nc.vector.tensor_copy(dh[:], hconv[li][0][o][:])
                            nc.sync.dma_start(dbg_h[li][o], dh[:])
                    for o in range(2):
                        dh3 = fo.tile([128, n_samples, L], FP32, tag=f"dbgh3{o}", name=f"dbgh3{o}")
                        nc.vector.tensor_copy(dh3[:], h3[o][:])
                        nc.sync.dma_start(dbg_h3[o], dh3[:])

    nc.compile()
    return nc


def prep_inputs(x, Wp, bp, W1, b1, W2, b2, W3, b3, Wfc1, bfc1, Wfc2, bfc2):
    """Host-side layout prep (numpy). Returns dict of full-size arrays keyed
    by the kernel's DRAM parameter names; 'xt' still has the full batch."""
    f32 = np.float32
    # x: (B, CTX*INST) -> (INST, B, CTX); bottom half = pos-0 col broadcast
    xt_top = np.ascontiguousarray(x.reshape(B, CTX, INST).transpose(2, 0, 1))
    xt_bot = np.broadcast_to(xt_top[:, :, 0:1], (INST, B, CTX))
    xt = np.concatenate([xt_top, xt_bot], axis=0).astype(BF16_NP)  # (128, B, CTX)
    # Wp: (PC, INST, 2) -> (128, PC): rows 0..63 = Wp[:,:,1].T, 64..127 = Wp[:,:,0].T
    wp = np.ascontiguousarray(
        np.concatenate([Wp[:, :, 1].T, Wp[:, :, 0].T], axis=0)
    ).astype(BF16_NP)
    # conv weights: (Cout, Cin, K) -> [cin_in, layer, cin_blk, k, cout_blk, cout_in]
    def conv_t(W):
        A = W.reshape(2, 128, 2, 128, 3)  # [ob, oi, ib, ii, k]
        return A.transpose(3, 2, 4, 0, 1)  # (128, 2, 3, 2, 128)

    wcv = np.ascontiguousarray(
        np.stack([conv_t(W1), conv_t(W2), conv_t(W3)], axis=1)
    ).astype(BF16_NP)
    # Wfc1: (400, 32512) with col = c3*L + l -> (L, cin_in, cin_blk, 400)
    wf1 = np.ascontiguousarray(
        Wfc1.reshape(F1, 2, 128, L).transpose(3, 2, 1, 0)
    ).astype(BF16_NP)
    bf1 = np.ascontiguousarray(bfc1.reshape(1, F1)).astype(BF16_NP)
    # Wfc2 (2, 400) replicated across partitions for the DVE fc2 reduce
    wf2 = np.ascontiguousarray(
        np.broadcast_to(Wfc2[None, :, :], (128, OUT, F1))
    ).astype(BF16_NP)
    bf2 = np.ascontiguousarray(np.broadcast_to(bfc2[None, :], (128, OUT))).astype(f32)
    # conv biases: (128, 8) fp32, col = layer*2 + blk
    bcv = np.ascontiguousarray(
        np.stack([bp, b1, b2, b3]).reshape(4, 2, 128).transpose(2, 0, 1).reshape(128, 8)
    ).astype(f32)
    return {
        "xt": xt,
        "wp": wp,
        "wcv": wcv,
        "wf1": wf1,
        "bf1": bf1,
        "wf2": wf2,
        "bcv": bcv,
        "bf2": bf2,
    }


_NC_CACHE = {}


def _get_nc(n_samples):
    if n_samples not in _NC_CACHE:
        _NC_CACHE[n_samples] = build_nc(n_samples)
    return _NC_CACHE[n_samples]


def run(inputs: dict, trace: bool = False, tmpdir: str | None = None):
    """Run on the 8 NeuronCores. Returns (output (B,2) fp32, exec_time_ns|None)."""
    full = prep_inputs(**inputs)
    xt = full.pop("xt")
    in_maps = []
    for c in range(N_CORES):
        m = dict(full)
        m["xt"] = np.ascontiguousarray(xt[:, c * BPC : (c + 1) * BPC, :])
        in_maps.append(m)
    nc = _get_nc(BPC)
    res = run_bass_kernel_spmd(
        nc,
        in_maps,
        list(range(N_CORES)),
        trace=trace,
        trace_cores=[0] if trace else None,
        tmpdir=tmpdir,
    )
    out = np.concatenate([np.asarray(r["out"]) for r in res.results], axis=0)
    return out.astype(np.float32), res.exec_time_ns


def kernel(**inputs) -> np.ndarray:
    return run(inputs, trace=False)[0]


# revision 12
# speedup vs baseline: 1.7266x; 1.1114x over previous
# Trainium2 Bass kernel for nn_CNN3_F_P (pairwise conv + 3x conv1d + 2 FC).
# Data parallel over 8 NeuronCores: batch 2048 -> 256 samples/core.
# Self-contained: hardcodes all shapes; host preps DMA-friendly weight layouts.
import sys

import numpy as np

try:
    import concourse.bass as bass  # noqa: F401
except ImportError:
    sys.path.insert(0, "/opt/trn_rl_repo")

import ml_dtypes

import concourse.bass as bass
import concourse.mybir as mybir
import concourse.tile as tile
from concourse import bacc
from concourse.bass_utils import run_bass_kernel_spmd

# Problem shapes
INST, CTX = 64, 128
PC = 256          # pairwise out channels; CH1=CH2=CH3=256
L = CTX - 1       # 127
F1, OUT = 400, 2
B = 2048
N_CORES = 8
BPC = B // N_CORES  # 256 samples per core
GT = 4              # samples per matmul group (free dim GT*L = 508 <= 512)

FP32 = mybir.dt.float32
BF16 = mybir.dt.bfloat16
BF16_NP = ml_dtypes.bfloat16
RELU = mybir.ActivationFunctionType.Relu
ADD = mybir.AluOpType.add
MULT = mybir.AluOpType.mult


def build_nc(n_samples: int, debug: bool = False) -> bass.Bass:
    """Emit the per-core Tile program. Every core runs this same program on
    its own 'n_samples'-sample shard."""
    assert n_samples % (2 * GT) == 0
    n_groups = n_samples // GT
    sb_n = n_samples // 128 if n_samples >= 128 else 1
    sb_sz = min(n_samples, 128)

    nc = bacc.Bacc()

    # DRAM parameters (per-core shard + replicated weights).
    # xt rows 0..63 = x transposed to (inst, sample, pos); rows 64..127 = the
    # pos-0 column broadcast along pos (so the pairwise layer is one matmul).
    xt_d = nc.declare_dram_parameter("xt", [128, n_samples, CTX], BF16, isOutput=False)
    wp_d = nc.declare_dram_parameter("wp", [128, PC], BF16, isOutput=False)
    wcv_d = nc.declare_dram_parameter("wcv", [128, 3, 2, 3, 2, 128], BF16, isOutput=False)
    wf1_d = nc.declare_dram_parameter("wf1", [L, 128, 2, F1], BF16, isOutput=False)
    bf1_d = nc.declare_dram_parameter("bf1", [1, F1], BF16, isOutput=False)
    wf2_d = nc.declare_dram_parameter("wf2", [128, OUT, F1], BF16, isOutput=False)
    bcv_d = nc.declare_dram_parameter("bcv", [128, 8], FP32, isOutput=False)
    bf2_d = nc.declare_dram_parameter("bf2", [128, OUT], FP32, isOutput=False)
    out_d = nc.declare_dram_parameter("out", [n_samples, OUT], FP32, isOutput=True)
    if debug:
        dbg_h = [
            nc.declare_dram_parameter(f"dbg_h{i}", [2, 128, GT, 129], FP32, isOutput=True)
            for i in range(3)
        ]
        dbg_h3 = nc.declare_dram_parameter("dbg_h3", [2, 128, n_samples, L], FP32, isOutput=True)
        dbg_f1 = nc.declare_dram_parameter("dbg_f1", [sb_n, 128, F1], FP32, isOutput=True)

    with tile.TileContext(nc) as tc:
        with (
            tc.tile_pool(name="consts", bufs=1) as consts,
            tc.tile_pool(name="hbuf", bufs=1) as hbuf,
            tc.tile_pool(name="xin", bufs=4) as xin,
        ):
            # ---- resident weights/biases ----
            wp_t = consts.tile([128, PC], BF16, tag="wp", name="wp")
            nc.sync.dma_start(wp_t[:], wp_d[:])
            bcv_t = consts.tile([128, 8], FP32, tag="bcv", name="bcv")
            nc.sync.dma_start(bcv_t[:], bcv_d[:])
            wcv_t = consts.tile([128, 3, 2, 3, 2, 128], BF16, tag="wcv", name="wcv")
            for li in range(3):
                nc.sync.dma_start(wcv_t[:, li], wcv_d[:, li])
            wf2_t = consts.tile([128, OUT, F1], BF16, tag="wf2", name="wf2")
            nc.sync.dma_start(wf2_t[:], wf2_d[:])
            bf1_t = consts.tile([1, F1], BF16, tag="bf1", name="bf1")
            nc.sync.dma_start(bf1_t[:], bf1_d[:])
            bf2_t = consts.tile([128, OUT], FP32, tag="bf2", name="bf2")
            nc.sync.dma_start(bf2_t[:], bf2_d[:])
            ones_t = consts.tile([1, 128], BF16, tag="ones", name="ones")
            nc.vector.memset(ones_t[:], 1.0)

            # ---- persistent activation buffers ----
            # h0..h2: ping-pong per group parity; stripes of 129 cols/sample
            # (col 0 and col 128 are zero pads for the k=3 conv taps).
            hconv = []  # hconv[layer][parity][blk]
            NPAR = [3, 2, 2]
            for layer in range(3):
                byp = []
                for par in range(NPAR[layer]):
                    blks = []
                    for o in range(2):
                        t = hbuf.tile(
                            [128, GT, 129], BF16,
                            tag=f"h{layer}_{par}_{o}", name=f"h{layer}_{par}_{o}",
                        )
                        nc.vector.memset(t[:, :, 0:1], 0.0)
                        nc.vector.memset(t[:, :, 128:129], 0.0)
                        blks.append(t)
                    byp.append(blks)
                hconv.append(byp)
            # h3: conv3 output for the whole shard, dense (no pads), bf16
            h3 = [
                hbuf.tile([128, n_samples, L], BF16, tag=f"h3_{o}", name=f"h3_{o}")
                for o in range(2)
            ]

            # ---- phase A: pairwise + conv1..conv3 ----
            # Pairwise for group g+2 is emitted ahead of group g's convs so the
            # PE never waits on the h0 relu; relus alternate Scalar (o=0) and
            # Vector (o=1) so both channel blocks finish in parallel.
            MAXALU = mybir.AluOpType.max

            def relu_to(dst, ps, bias_idx, use_dve):
                if use_dve:
                    nc.vector.tensor_scalar(
                        dst, ps, bcv_t[:, bias_idx : bias_idx + 1], 0.0, ADD, MAXALU
                    )
                else:
                    nc.scalar.activation(
                        dst, ps, RELU, bias=bcv_t[:, bias_idx : bias_idx + 1]
                    )

            with (
                tc.tile_pool(name="cpsum", bufs=4, space=bass.MemorySpace.PSUM) as cp,
                tc.tile_pool(name="ppsum", bufs=4, space=bass.MemorySpace.PSUM) as pp,
            ):

                def pairwise(g):
                    par = g % 3
                    s0 = g * GT
                    px = xin.tile([128, GT, CTX], BF16, tag="px", name="px")
                    nc.sync.dma_start(px[:], xt_d[:, s0 : s0 + GT, :])
                    for o in range(2):
                        ps = pp.tile([128, GT, L], FP32, tag="pp", name="pp")
                        nc.tensor.matmul(
                            ps[:],
                            wp_t[:, o * 128 : (o + 1) * 128],
                            px[:, :, 1:CTX],
                            start=True,
                            stop=True,
                        )
                        relu_to(hconv[0][par][o][:, :, 1:128], ps[:], o, o == 1)

                def conv_layer(li, g):
                    rpar = g % 3 if li == 0 else g % 2
                    wpar = g % 2
                    s0 = g * GT
                    for o in range(2):
                        ps = cp.tile([128, GT, L], FP32, tag="cp", name="cp")
                        n_mm = 0
                        for i in range(2):
                            for k in range(3):
                                nc.tensor.matmul(
                                    ps[:],
                                    wcv_t[:, li, i, k, o, :],
                                    hconv[li][rpar][i][:, :, k : k + L],
                                    start=(n_mm == 0),
                                    stop=(n_mm == 5),
                                )
                                n_mm += 1
                        if li < 2:
                            dst = hconv[li + 1][wpar][o][:, :, 1:128]
                        else:
                            dst = h3[o][:, s0 : s0 + GT, :]
                        relu_to(dst, ps[:], 2 * (li + 1) + o, o == 1)

                pairwise(0)
                if n_groups > 1:
                    pairwise(1)
                for g in range(n_groups):
                    if g + 2 < n_groups:
                        pairwise(g + 2)
                    for li in range(3):
                        conv_layer(li, g)

            # ---- phase B: fc1 (+relu) and fc2 ----
            # fc1 runs "flipped": stationary = h3 sample-block columns,
            # moving = streamed Wfc1 rows -> psum[sample, f1].
            with (
                tc.tile_pool(name="fpsum", bufs=1, space=bass.MemorySpace.PSUM) as fp,
                tc.tile_pool(name="wstream", bufs=12) as ws,
                tc.tile_pool(name="fout", bufs=1) as fo,
            ):
                f1ps = [
                    fp.tile([sb_sz, F1], FP32, tag=f"f1p{sb}", name=f"f1p{sb}")
                    for sb in range(sb_n)
                ]
                # bias row via a K=1 matmul of ones^T x bfc1
                for sb in range(sb_n):
                    nc.tensor.matmul(
                        f1ps[sb][:],
                        ones_t[:, :sb_sz],
                        bf1_t[:],
                        start=True,
                        stop=False,
                    )
                for l in range(L):
                    wt = ws.tile([128, 2, F1], BF16, tag="wf1", name="wf1_t")
                    nc.sync.dma_start(wt[:], wf1_d[l])
                    for i in range(2):
                        for sb in range(sb_n):
                            nc.tensor.matmul(
                                f1ps[sb][:],
                                h3[i][:, sb * 128 : sb * 128 + sb_sz, l],
                                wt[:, i, :],
                                start=False,
                                stop=(l == L - 1 and i == 1),
                            )
                for sb in range(sb_n):
                    f1o = fo.tile([sb_sz, F1], BF16, tag=f"f1o{sb}", name=f"f1o{sb}")
                    nc.scalar.activation(f1o[:], f1ps[sb][:], RELU)
                    out_t = fo.tile([sb_sz, OUT], FP32, tag=f"out{sb}", name=f"out{sb}")
                    for o in range(OUT):
                        tmp = fo.tile([sb_sz, F1], FP32, tag="tmp", name="tmp", bufs=2)
                        nc.vector.tensor_tensor(tmp[:], f1o[:], wf2_t[:sb_sz, o, :], MULT)
                        nc.vector.tensor_reduce(
                            out_t[:, o : o + 1], tmp[:], mybir.AxisListType.X, ADD
                        )
                    nc.vector.tensor_tensor(out_t[:], out_t[:], bf2_t[:sb_sz, :], ADD)
                    nc.sync.dma_start(out_d[sb * 128 : sb * 128 + sb_sz, :], out_t[:])
                    if debug:
                        df = fo.tile([sb_sz, F1], FP32, tag=f"dbgf{sb}", name=f"dbgf{sb}")
                        nc.vector.tensor_copy(df[:], f1o[:])
                        nc.sync.dma_start(dbg_f1[sb, :sb_sz, :], df[:])
                if debug:
                    for li in range(3):
                        for o in range(2):
                            dh = fo.tile([128, GT, 129], FP32, tag=f"dbg{li}{o}", name=f"dbg{li}{o}")
                            nc.vector.tensor_copy(dh[:], hconv[li][0][o][:])
                            nc.sync.dma_start(dbg_h[li][o], dh[:])
                    for o in range(2):
                        dh3 = fo.tile([128, n_samples, L], FP32, tag=f"dbgh3{o}", name=f"dbgh3{o}")
                        nc.vector.tensor_copy(dh3[:], h3[o][:])
                        nc.sync.dma_start(dbg_h3[o], dh3[:])

    nc.compile()
    return nc


def prep_inputs(x, Wp, bp, W1, b1, W2, b2, W3, b3, Wfc1, bfc1, Wfc2, bfc2):
    """Host-side layout prep (numpy). Returns dict of full-size arrays keyed
    by the kernel's DRAM parameter names; 'xt' still has the full batch."""
    f32 = np.float32
    # x: (B, CTX*INST) -> (INST, B, CTX); bottom half = pos-0 col broadcast
    xt_top = np.ascontiguousarray(x.reshape(B, CTX, INST).transpose(2, 0, 1))
    xt_bot = np.broadcast_to(xt_top[:, :, 0:1], (INST, B, CTX))
    xt = np.concatenate([xt_top, xt_bot], axis=0).astype(BF16_NP)  # (128, B, CTX)
    # Wp: (PC, INST, 2) -> (128, PC): rows 0..63 = Wp[:,:,1].T, 64..127 = Wp[:,:,0].T
    wp = np.ascontiguousarray(
        np.concatenate([Wp[:, :, 1].T, Wp[:, :, 0].T], axis=0)
    ).astype(BF16_NP)
    # conv weights: (Cout, Cin, K) -> [cin_in, layer, cin_blk, k, cout_blk, cout_in]
    def conv_t(W):
        A = W.reshape(2, 128, 2, 128, 3)  # [ob, oi, ib, ii, k]
        return A.transpose(3, 2, 4, 0, 1)  # (128, 2, 3, 2, 128)

    wcv = np.ascontiguousarray(
        np.stack([conv_t(W1), conv_t(W2), conv_t(W3)], axis=1)
    ).astype(BF16_NP)
    # Wfc1: (400, 32512) with col = c3*L + l -> (L, cin_in, cin_blk, 400)
    wf1 = np.ascontiguousarray(
        Wfc1.reshape(F1, 2, 128, L).transpose(3, 2, 1, 0)
    ).astype(BF16_NP)
    bf1 = np.ascontiguousarray(bfc1.reshape(1, F1)).astype(BF16_NP)
    # Wfc2 (2, 400) replicated across partitions for the DVE fc2 reduce
    wf2 = np.ascontiguousarray(
        np.broadcast_to(Wfc2[None, :, :], (128, OUT, F1))
    ).astype(BF16_NP)
    bf2 = np.ascontiguousarray(np.broadcast_to(bfc2[None, :], (128, OUT))).astype(f32)
    # conv biases: (128, 8) fp32, col = layer*2 + blk
    bcv = np.ascontiguousarray(
        np.stack([bp, b1, b2, b3]).reshape(4, 2, 128).transpose(2, 0, 1).reshape(128, 8)
    ).astype(f32)
    return {
        "xt": xt,
        "wp": wp,
        "wcv": wcv,
        "wf1": wf1,
        "bf1": bf1,
        "wf2": wf2,
        "bcv": bcv,
        "bf2": bf2,
    }


_NC_CACHE = {}


def _get_nc(n_samples):
    if n_samples not in _NC_CACHE:
        _NC_CACHE[n_samples] = build_nc(n_samples)
    return _NC_CACHE[n_samples]


def run(inputs: dict, trace: bool = False, tmpdir: str | None = None):
    """Run on the 8 NeuronCores. Returns (output (B,2) fp32, exec_time_ns|None)."""
    full = prep_inputs(**inputs)
    xt = full.pop("xt")
    in_maps = []
    for c in range(N_CORES):
        m = dict(full)
        m["xt"] = np.ascontiguousarray(xt[:, c * BPC : (c + 1) * BPC, :])
        in_maps.append(m)
    nc = _get_nc(BPC)
    res = run_bass_kernel_spmd(
        nc,
        in_maps,
        list(range(N_CORES)),
        trace=trace,
        trace_cores=[0] if trace else None,
        tmpdir=tmpdir,
    )
    out = np.concatenate([np.asarray(r["out"]) for r in res.results], axis=0)
    return out.astype(np.float32), res.exec_time_ns


def kernel(**inputs) -> np.ndarray:
    return run(inputs, trace=False)[0]


# revision 13
# speedup vs baseline: 1.7350x; 1.0049x over previous
# Trainium2 Bass kernel for nn_CNN3_F_P (pairwise conv + 3x conv1d + 2 FC).
# Data parallel over 8 NeuronCores: batch 2048 -> 256 samples/core.
# Self-contained: hardcodes all shapes; host preps DMA-friendly weight layouts.
import sys

import numpy as np

try:
    import concourse.bass as bass  # noqa: F401
except ImportError:
    sys.path.insert(0, "/opt/trn_rl_repo")

import ml_dtypes

import concourse.bass as bass
import concourse.mybir as mybir
import concourse.tile as tile
from concourse import bacc
from concourse.bass_utils import run_bass_kernel_spmd

# Problem shapes
INST, CTX = 64, 128
PC = 256          # pairwise out channels; CH1=CH2=CH3=256
L = CTX - 1       # 127
F1, OUT = 400, 2
B = 2048
N_CORES = 8
BPC = B // N_CORES  # 256 samples per core
GT = 4              # samples per matmul group (free dim GT*L = 508 <= 512)

FP32 = mybir.dt.float32
BF16 = mybir.dt.bfloat16
BF16_NP = ml_dtypes.bfloat16
RELU = mybir.ActivationFunctionType.Relu
ADD = mybir.AluOpType.add
MULT = mybir.AluOpType.mult


def build_nc(n_samples: int, debug: bool = False) -> bass.Bass:
    """Emit the per-core Tile program. Every core runs this same program on
    its own 'n_samples'-sample shard."""
    assert n_samples % (2 * GT) == 0
    n_groups = n_samples // GT
    sb_n = n_samples // 128 if n_samples >= 128 else 1
    sb_sz = min(n_samples, 128)

    nc = bacc.Bacc()

    # DRAM parameters (per-core shard + replicated weights).
    # xt rows 0..63 = x transposed to (inst, sample, pos); rows 64..127 = the
    # pos-0 column broadcast along pos (so the pairwise layer is one matmul).
    xt_d = nc.declare_dram_parameter("xt", [128, n_samples, CTX], BF16, isOutput=False)
    wp_d = nc.declare_dram_parameter("wp", [128, PC], BF16, isOutput=False)
    wcv_d = nc.declare_dram_parameter("wcv", [128, 3, 2, 3, 2, 128], BF16, isOutput=False)
    wf1_d = nc.declare_dram_parameter("wf1", [L, 128, 2, F1], BF16, isOutput=False)
    bf1_d = nc.declare_dram_parameter("bf1", [1, F1], BF16, isOutput=False)
    wf2_d = nc.declare_dram_parameter("wf2", [128, OUT, F1], BF16, isOutput=False)
    bcv_d = nc.declare_dram_parameter("bcv", [128, 8], FP32, isOutput=False)
    bf2_d = nc.declare_dram_parameter("bf2", [128, OUT], FP32, isOutput=False)
    out_d = nc.declare_dram_parameter("out", [n_samples, OUT], FP32, isOutput=True)
    if debug:
        dbg_h = [
            nc.declare_dram_parameter(f"dbg_h{i}", [2, 128, GT, 129], FP32, isOutput=True)
            for i in range(3)
        ]
        dbg_h3 = nc.declare_dram_parameter("dbg_h3", [2, 128, n_samples, L], FP32, isOutput=True)
        dbg_f1 = nc.declare_dram_parameter("dbg_f1", [sb_n, 128, F1], FP32, isOutput=True)

    with tile.TileContext(nc) as tc:
        with (
            tc.tile_pool(name="consts", bufs=1) as consts,
            tc.tile_pool(name="hbuf", bufs=1) as hbuf,
            tc.tile_pool(name="xin", bufs=4) as xin,
        ):
            # ---- resident weights/biases ----
            wp_t = consts.tile([128, PC], BF16, tag="wp", name="wp")
            nc.sync.dma_start(wp_t[:], wp_d[:])
            bcv_t = consts.tile([128, 8], FP32, tag="bcv", name="bcv")
            nc.sync.dma_start(bcv_t[:], bcv_d[:])
            wcv_t = consts.tile([128, 3, 2, 3, 2, 128], BF16, tag="wcv", name="wcv")
            wf2_t = consts.tile([128, OUT, F1], BF16, tag="wf2", name="wf2")
            bf1_t = consts.tile([1, F1], BF16, tag="bf1", name="bf1")
            bf2_t = consts.tile([128, OUT], FP32, tag="bf2", name="bf2")
            ones_t = consts.tile([1, 128], BF16, tag="ones", name="ones")
            nc.vector.memset(ones_t[:], 1.0)

            # ---- persistent activation buffers ----
            # h0..h2: ping-pong per group parity; stripes of 129 cols/sample
            # (col 0 and col 128 are zero pads for the k=3 conv taps).
            hconv = []  # hconv[layer][parity][blk]
            NPAR = [3, 2, 2]
            for layer in range(3):
                byp = []
                for par in range(NPAR[layer]):
                    blks = []
                    for o in range(2):
                        t = hbuf.tile(
                            [128, GT, 129], BF16,
                            tag=f"h{layer}_{par}_{o}", name=f"h{layer}_{par}_{o}",
                        )
                        nc.vector.memset(t[:, :, 0:1], 0.0)
                        nc.vector.memset(t[:, :, 128:129], 0.0)
                        blks.append(t)
                    byp.append(blks)
                hconv.append(byp)
            # h3: conv3 output for the whole shard, dense (no pads), bf16
            h3 = [
                hbuf.tile([128, n_samples, L], BF16, tag=f"h3_{o}", name=f"h3_{o}")
                for o in range(2)
            ]

            # ---- phase A: pairwise + conv1..conv3 ----
            # Pairwise for group g+2 is emitted ahead of group g's convs so the
            # PE never waits on the h0 relu; relus alternate Scalar (o=0) and
            # Vector (o=1) so both channel blocks finish in parallel.
            MAXALU = mybir.AluOpType.max

            def relu_to(dst, ps, bias_idx, use_dve):
                if use_dve:
                    nc.vector.tensor_scalar(
                        dst, ps, bcv_t[:, bias_idx : bias_idx + 1], 0.0, ADD, MAXALU
                    )
                else:
                    nc.scalar.activation(
                        dst, ps, RELU, bias=bcv_t[:, bias_idx : bias_idx + 1]
                    )

            with (
                tc.tile_pool(name="cpsum", bufs=4, space=bass.MemorySpace.PSUM) as cp,
                tc.tile_pool(name="ppsum", bufs=4, space=bass.MemorySpace.PSUM) as pp,
            ):

                def pairwise(g):
                    par = g % 3
                    s0 = g * GT
                    px = xin.tile([128, GT, CTX], BF16, tag="px", name="px")
                    nc.sync.dma_start(px[:], xt_d[:, s0 : s0 + GT, :])
                    for o in range(2):
                        ps = pp.tile([128, GT, L], FP32, tag="pp", name="pp")
                        nc.tensor.matmul(
                            ps[:],
                            wp_t[:, o * 128 : (o + 1) * 128],
                            px[:, :, 1:CTX],
                            start=True,
                            stop=True,
                        )
                        relu_to(hconv[0][par][o][:, :, 1:128], ps[:], o, o == 1)

                def conv_layer(li, g):
                    rpar = g % 3 if li == 0 else g % 2
                    wpar = g % 2
                    s0 = g * GT
                    for o in range(2):
                        ps = cp.tile([128, GT, L], FP32, tag="cp", name="cp")
                        n_mm = 0
                        for i in range(2):
                            for k in range(3):
                                nc.tensor.matmul(
                                    ps[:],
                                    wcv_t[:, li, i, k, o, :],
                                    hconv[li][rpar][i][:, :, k : k + L],
                                    start=(n_mm == 0),
                                    stop=(n_mm == 5),
                                )
                                n_mm += 1
                        if li < 2:
                            dst = hconv[li + 1][wpar][o][:, :, 1:128]
                        else:
                            dst = h3[o][:, s0 : s0 + GT, :]
                        relu_to(dst, ps[:], 2 * (li + 1) + o, o == 1)

                pairwise(0)
                if n_groups > 1:
                    pairwise(1)
                # big consts stream in behind the first input tiles
                for li in range(3):
                    nc.sync.dma_start(wcv_t[:, li], wcv_d[:, li])
                nc.sync.dma_start(wf2_t[:], wf2_d[:])
                nc.sync.dma_start(bf1_t[:], bf1_d[:])
                nc.sync.dma_start(bf2_t[:], bf2_d[:])
                for g in range(n_groups):
                    if g + 2 < n_groups:
                        pairwise(g + 2)
                    for li in range(3):
                        conv_layer(li, g)

            # ---- phase B: fc1 (+relu) and fc2 ----
            # fc1 runs "flipped": stationary = h3 sample-block columns,
            # moving = streamed Wfc1 rows -> psum[sample, f1].
            with (
                tc.tile_pool(name="fpsum", bufs=1, space=bass.MemorySpace.PSUM) as fp,
                tc.tile_pool(name="wstream", bufs=12) as ws,
                tc.tile_pool(name="fout", bufs=1) as fo,
            ):
                f1ps = [
                    fp.tile([sb_sz, F1], FP32, tag=f"f1p{sb}", name=f"f1p{sb}")
                    for sb in range(sb_n)
                ]
                # bias row via a K=1 matmul of ones^T x bfc1
                for sb in range(sb_n):
                    nc.tensor.matmul(
                        f1ps[sb][:],
                        ones_t[:, :sb_sz],
                        bf1_t[:],
                        start=True,
                        stop=False,
                    )
                for l in range(L):
                    wt = ws.tile([128, 2, F1], BF16, tag="wf1", name="wf1_t")
                    nc.sync.dma_start(wt[:], wf1_d[l])
                    for i in range(2):
                        for sb in range(sb_n):
                            nc.tensor.matmul(
                                f1ps[sb][:],
                                h3[i][:, sb * 128 : sb * 128 + sb_sz, l],
                                wt[:, i, :],
                                start=False,
                                stop=(l == L - 1 and i == 1),
                            )
                for sb in range(sb_n):
                    f1o = fo.tile([sb_sz, F1], BF16, tag=f"f1o{sb}", name=f"f1o{sb}")
                    nc.scalar.activation(f1o[:], f1ps[sb][:], RELU)
                    out_t = fo.tile([sb_sz, OUT], FP32, tag=f"out{sb}", name=f"out{sb}")
                    for o in range(OUT):
                        tmp = fo.tile([sb_sz, F1], FP32, tag="tmp", name="tmp", bufs=2)
                        nc.vector.tensor_tensor(tmp[:], f1o[:], wf2_t[:sb_sz, o, :], MULT)
                        nc.vector.tensor_reduce(
                            out_t[:, o : o + 1], tmp[:], mybir.AxisListType.X, ADD
                        )
                    nc.vector.tensor_tensor(out_t[:], out_t[:], bf2_t[:sb_sz, :], ADD)
                    nc.sync.dma_start(out_d[sb * 128 : sb * 128 + sb_sz, :], out_t[:])
                    if debug:
                        df = fo.tile([sb_sz, F1], FP32, tag=f"dbgf{sb}", name=f"dbgf{sb}")
                        nc.vector.tensor_copy(df[:], f1o[:])
                        nc.sync.dma_start(dbg_f1[sb, :sb_sz, :], df[:])
                if debug:
                    for li in range(3):
                        for o in range(2):
                            dh = fo.tile([128, GT, 129], FP32, tag=f"dbg{li}{o}", name=f"dbg{li}{o}")
                            nc.vector.tensor_copy(dh[:], hconv[li][0][o][:])
                            nc.sync.dma_start(dbg_h[li][o], dh[:])
                    for o in range(2):
                        dh3 = fo.tile([128, n_samples, L], FP32, tag=f"dbgh3{o}", name=f"dbgh3{o}")
                        nc.vector.tensor_copy(dh3[:], h3[o][:])
                        nc.sync.dma_start(dbg_h3[o], dh3[:])

    nc.compile()
    return nc


def prep_inputs(x, Wp, bp, W1, b1, W2, b2, W3, b3, Wfc1, bfc1, Wfc2, bfc2):
    """Host-side layout prep (numpy). Returns dict of full-size arrays keyed
    by the kernel's DRAM parameter names; 'xt' still has the full batch."""
    f32 = np.float32
    # x: (B, CTX*INST) -> (INST, B, CTX); bottom half = pos-0 col broadcast
    xt_top = np.ascontiguousarray(x.reshape(B, CTX, INST).transpose(2, 0, 1))
    xt_bot = np.broadcast_to(xt_top[:, :, 0:1], (INST, B, CTX))
    xt = np.concatenate([xt_top, xt_bot], axis=0).astype(BF16_NP)  # (128, B, CTX)
    # Wp: (PC, INST, 2) -> (128, PC): rows 0..63 = Wp[:,:,1].T, 64..127 = Wp[:,:,0].T
    wp = np.ascontiguousarray(
        np.concatenate([Wp[:, :, 1].T, Wp[:, :, 0].T], axis=0)
    ).astype(BF16_NP)
    # conv weights: (Cout, Cin, K) -> [cin_in, layer, cin_blk, k, cout_blk, cout_in]
    def conv_t(W):
        A = W.reshape(2, 128, 2, 128, 3)  # [ob, oi, ib, ii, k]
        return A.transpose(3, 2, 4, 0, 1)  # (128, 2, 3, 2, 128)

    wcv = np.ascontiguousarray(
        np.stack([conv_t(W1), conv_t(W2), conv_t(W3)], axis=1)
    ).astype(BF16_NP)
    # Wfc1: (400, 32512) with col = c3*L + l -> (L, cin_in, cin_blk, 400)
    wf1 = np.ascontiguousarray(
        Wfc1.reshape(F1, 2, 128, L).transpose(3, 2, 1, 0)
    ).astype(BF16_NP)
    bf1 = np.ascontiguousarray(bfc1.reshape(1, F1)).astype(BF16_NP)
    # Wfc2 (2, 400) replicated across partitions for the DVE fc2 reduce
    wf2 = np.ascontiguousarray(
        np.broadcast_to(Wfc2[None, :, :], (128, OUT, F1))
    ).astype(BF16_NP)
    bf2 = np.ascontiguousarray(np.broadcast_to(bfc2[None, :], (128, OUT))).astype(f32)
    # conv biases: (128, 8) fp32, col = layer*2 + blk
    bcv = np.ascontiguousarray(
        np.stack([bp, b1, b2, b3]).reshape(4, 2, 128).transpose(2, 0, 1).reshape(128, 8)
    ).astype(f32)
    return {
        "xt": xt,
        "wp": wp,
        "wcv": wcv,
        "wf1": wf1,
        "bf1": bf1,
        "wf2": wf2,
        "bcv": bcv,
        "bf2": bf2,
    }


_NC_CACHE = {}


def _get_nc(n_samples):
    if n_samples not in _NC_CACHE:
        _NC_CACHE[n_samples] = build_nc(n_samples)
    return _NC_CACHE[n_samples]


def run(inputs: dict, trace: bool = False, tmpdir: str | None = None):
    """Run on the 8 NeuronCores. Returns (output (B,2) fp32, exec_time_ns|None)."""
    full = prep_inputs(**inputs)
    xt = full.pop("xt")
    in_maps = []
    for c in range(N_CORES):
        m = dict(full)
        m["xt"] = np.ascontiguousarray(xt[:, c * BPC : (c + 1) * BPC, :])
        in_maps.append(m)
    nc = _get_nc(BPC)
    res = run_bass_kernel_spmd(
        nc,
        in_maps,
        list(range(N_CORES)),
        trace=trace,
        trace_cores=[0] if trace else None,
        tmpdir=tmpdir,
    )
    out = np.concatenate([np.asarray(r["out"]) for r in res.results], axis=0)
    return out.astype(np.float32), res.exec_time_ns


def kernel(**inputs) -> np.ndarray:
    return run(inputs, trace=False)[0]
